# revision 1
# baseline (speedup 1.0000x reference)
"""CrossViewAttention Trainium2 Bass kernel.

Math (per batch-group b of NV=8 views, identical to reference):
  kmean[b,j]   = mean_s(x[b,j,s,:]) @ Wk + bk            (linearity of mean)
  scores       = (x @ Wq + bq) . kmean * scale
               = x @ Wqm[b] + bqm[b]                      (fold Wq into kmean)
  w            = softmax_j(scores)
  out[b,i,s]   = sum_j w[b,i,j,h,s] * v[b,j,s,(h,:)],  v = x @ Wv   (+bv folded)
  y            = out @ Wo + (bv @ Wo + bo)                (softmax sums to 1)

Sharding: 8 cores = 4 batch-groups x 2 S-halves. The only cross-core
quantity is the per-(b,view) feature-sum of x over S, exchanged with a
tiny AllReduce ([4,8,512] f32, full 8-core group with one-hot masking so
the SPMD program needs no core-dependent addressing).

Device layout: x is loaded row-major, cast, and PE-transposed into
xT[d,row] tiles so TensorE can contract over D.  v/scores/y matmuls keep
rows on PSUM partitions.  The cross-view mix is elementwise on VectorE
(per-position 8x8 view mixing is not expressible as a matmul).
"""

import os
import numpy as np
from contextlib import ExitStack

import concourse.bass as bass
import concourse.bacc as bacc
import concourse.tile as tile
import concourse.mybir as mybir
from concourse.bass_utils import run_bass_kernel_spmd
from concourse.masks import make_identity

# ---- problem constants (hardcoded; kernel.py must be self-contained) ----
B, NV, S, D, H, HD = 4, 8, 2048, 512, 8, 64
NCORES = 8
# CVA_SPC: shrink per-core positions for simulator debugging only
SPC = int(os.environ.get("CVA_SPC", S // 2))
SH = SPC // 128         # s-blocks of 128
T = D // 128            # 4 contraction tiles
SCALE = HD ** -0.5

F32 = mybir.dt.float32
BF16 = mybir.dt.bfloat16
F32R = mybir.dt.float32r

REPLICA_GROUPS = [list(range(NCORES))]


def _cfg():
    return dict(
        mix_impl=os.environ.get("CVA_MIX", "tt2"),        # tt2 | stt
        mm_dt=os.environ.get("CVA_MM_DT", "bf16"),        # bf16 | f32r | f32
        mix_dt=os.environ.get("CVA_MIX_DT", "bf16"),      # bf16 | f32
        reps=int(os.environ.get("CVA_REPS", "1")),        # timing: body reps
    )


def _mmdt(cfg):
    # storage dtype for matmul operands
    return BF16 if cfg["mm_dt"] == "bf16" else F32


def _mm_ap(ap, cfg):
    # bitcast f32 storage to f32r at matmul sites when requested
    if cfg["mm_dt"] == "f32r":
        return ap.bitcast(F32R)
    return ap


def build_kernel(cfg):
    nc = bacc.Bacc(
        "TRN2", target_bir_lowering=False, debug=False, num_devices=NCORES
    )
    mm_dt = _mmdt(cfg)
    mix_dt = BF16 if cfg["mix_dt"] == "bf16" else F32

    x = nc.dram_tensor("x", [NV, SPC, D], F32, kind="ExternalInput").ap()
    Wq = nc.dram_tensor("Wq", [D, D], F32, kind="ExternalInput").ap()
    bq = nc.dram_tensor("bq", [D], F32, kind="ExternalInput").ap()
    Wk = nc.dram_tensor("Wk", [D, D], F32, kind="ExternalInput").ap()
    bk = nc.dram_tensor("bk", [D], F32, kind="ExternalInput").ap()
    Wv = nc.dram_tensor("Wv", [D, D], F32, kind="ExternalInput").ap()
    bv = nc.dram_tensor("bv", [D], F32, kind="ExternalInput").ap()
    Wo = nc.dram_tensor("Wo", [D, D], F32, kind="ExternalInput").ap()
    bo = nc.dram_tensor("bo", [D], F32, kind="ExternalInput").ap()
    mask = nc.dram_tensor("mask", [128, B], F32, kind="ExternalInput").ap()
    y = nc.dram_tensor("y", [NV, SPC, D], F32, kind="ExternalOutput").ap()

    with tile.TileContext(nc) as tc:
        for _rep in range(cfg.get("reps", 1)):
            _body(tc, cfg, mm_dt, mix_dt, x, Wq, bq, Wk, bk, Wv, bv, Wo, bo,
                  mask, y)

    nc.compile()
    return nc


def _body(tc, cfg, mm_dt, mix_dt, x, Wq, bq, Wk, bk, Wv, bv, Wo, bo, mask, y):
    nc = tc.nc
    Exp = mybir.ActivationFunctionType.Exp
    Copy = mybir.ActivationFunctionType.Copy
    ADD = mybir.AluOpType.add
    MULT = mybir.AluOpType.mult

    ctx = ExitStack()
    with ctx:
        consts = ctx.enter_context(tc.tile_pool(name="consts", bufs=1))

        # ---- constants / weights resident in SBUF ----
        ident_f32 = consts.tile([128, 128], F32)
        make_identity(nc, ident_f32[:, :])
        if BF16 in (mm_dt, mix_dt):
            ident_bf16 = consts.tile([128, 128], BF16)
            make_identity(nc, ident_bf16[:, :])
        else:
            ident_bf16 = ident_f32
        ident_mm = ident_bf16 if mm_dt == BF16 else ident_f32
        ident_mix = ident_bf16 if mix_dt == BF16 else ident_f32
        ones_mm = consts.tile([1, 128], mm_dt)
        nc.vector.memset(ones_mm[:, :], 1.0)
        ones128 = consts.tile([128, 1], mm_dt)
        nc.vector.memset(ones128[:, :], 1.0)
        # onehots[:, i, :] = column-i one-hot [128, NV] (xsum stationary)
        onehots = consts.tile([128, NV, NV], mm_dt)
        nc.vector.memset(onehots[:, :, :], 0.0)
        for i in range(NV):
            nc.vector.memset(onehots[:, i, i : i + 1], 1.0)
        one1_f32 = consts.tile([1, 1], F32)
        nc.vector.memset(one1_f32[:, :], 1.0)

        # moving-operand weights [d_in(128), t, d_out(512)]
        wv_sb = consts.tile([128, T, D], mm_dt)
        wo_sb = consts.tile([128, T, D], mm_dt)
        wsrc = {BF16: nc.gpsimd, F32: nc.sync}[mm_dt]
        wsrc.dma_start(
            out=wv_sb[:, :, :], in_=Wv.rearrange("(t p) n -> p t n", p=128)
        )
        wsrc.dma_start(
            out=wo_sb[:, :, :], in_=Wo.rearrange("(t p) n -> p t n", p=128)
        )
        wk_sb = consts.tile([128, T, D], F32)
        nc.sync.dma_start(
            out=wk_sb[:, :, :], in_=Wk.rearrange("(t p) n -> p t n", p=128)
        )
        wq_sb = consts.tile([128, T, D], F32)
        nc.sync.dma_start(
            out=wq_sb[:, :, :], in_=Wq.rearrange("(t p) n -> p t n", p=128)
        )
        bk1 = consts.tile([1, D], F32)
        nc.sync.dma_start(out=bk1[:, :], in_=bk.unsqueeze(0))
        bo1 = consts.tile([1, D], F32)
        nc.sync.dma_start(out=bo1[:, :], in_=bo.unsqueeze(0))
        bqT = consts.tile([128, T], F32)
        nc.sync.dma_start(out=bqT[:, :], in_=bq.rearrange("(t p) -> p t", p=128))
        bvT = consts.tile([128, T], mm_dt)
        wsrc.dma_start(out=bvT[:, :], in_=bv.rearrange("(t p) -> p t", p=128))
        mask_sb = consts.tile([128, B], F32)
        nc.sync.dma_start(out=mask_sb[:, :], in_=mask)

        # resident row-major staged activations [s%128, view, s_blk, d]
        stg = consts.tile([128, NV, SH, D], mm_dt)

        # ---- phase 1: load x + per-view feature sums ----
        ph1 = ExitStack()
        with ph1:
            xs_ps = ph1.enter_context(
                tc.tile_pool(name="xs_ps", bufs=1, space="PSUM")
            )
            xsum_ps = xs_ps.tile([NV, D], F32)
            for i in range(NV):
                if mm_dt == BF16:
                    nc.gpsimd.dma_start(
                        out=stg[:, i, :, :],
                        in_=x[i].rearrange("(sh p) d -> p sh d", p=128),
                    )
                else:
                    nc.sync.dma_start(
                        out=stg[:, i, :, :],
                        in_=x[i].rearrange("(sh p) d -> p sh d", p=128),
                    )
                # feature sums via one-hot matmul; one accumulation group
                # over all views and s-blocks into xsum_ps[view, :]
                for sh in range(SH):
                    nc.tensor.matmul(
                        xsum_ps[:, :],
                        _mm_ap(onehots[:, i, :], cfg),
                        _mm_ap(stg[:, i, sh, :], cfg),
                        start=(i == 0 and sh == 0),
                        stop=(i == NV - 1 and sh == SH - 1),
                    )
            xsum_sb = consts.tile([NV, D], F32)
            nc.scalar.copy(out=xsum_sb[:, :], in_=xsum_ps[:, :])

        # ---- phase 2: exchange partial sums (masked full-group AllReduce) ----
        ph2 = ExitStack()
        with ph2:
            dram = ph2.enter_context(
                tc.tile_pool(name="dram", bufs=1, space="DRAM")
            )
            sb2 = ph2.enter_context(tc.tile_pool(name="sb2", bufs=1))
            xsum4 = sb2.tile([NV, B, D], F32)
            for bb in range(B):
                nc.vector.tensor_scalar(
                    xsum4[:, bb, :],
                    xsum_sb[:, :],
                    mask_sb[0:NV, bb : bb + 1],
                    None,
                    op0=MULT,
                )
            cc_in = dram.tile([B, NV, D], F32)
            cc_out = dram.tile([B, NV, D], F32, addr_space="Shared")
            nc.sync.dma_start(
                out=cc_in[:, :, :].rearrange("b j d -> j b d"),
                in_=xsum4[:, :, :],
            )
            nc.gpsimd.collective_compute(
                "AllReduce",
                ADD,
                replica_groups=REPLICA_GROUPS,
                ins=[cc_in[:, :, :]],
                outs=[cc_out[:, :, :]],
            )
            # pull back all 4 groups, mask-select ours, scale by 1/S * scale
            xsf4 = sb2.tile([128, B, T, NV], F32)
            for bb in range(B):
                for t in range(T):
                    nc.sync.dma_start(
                        out=xsf4[:, bb, t, :],
                        in_=cc_out[bb, :, t * 128 : (t + 1) * 128].rearrange(
                            "j p -> p j"
                        ),
                    )
            xsf = sb2.tile([128, T, NV], F32)
            nc.vector.tensor_scalar(
                xsf[:, :, :],
                xsf4[:, 0, :, :],
                mask_sb[:, 0:1],
                None,
                op0=MULT,
            )
            for bb in range(1, B):
                nc.vector.scalar_tensor_tensor(
                    out=xsf[:, :, :],
                    in0=xsf4[:, bb, :, :],
                    scalar=mask_sb[:, bb : bb + 1],
                    in1=xsf[:, :, :],
                    op0=MULT,
                    op1=ADD,
                )
            nc.vector.tensor_scalar(
                xsf[:, :, :], xsf[:, :, :], SCALE / (2 * SPC), None, op0=MULT
            )

            # ---- phase 3: kmeanT, Wqm, bqm, bo' ----
            # kmT[:, to, j] = kmean[j, to*128+p] * SCALE
            #   = (xsum[j]*SCALE/S) @ Wk[:, to-block] + bk[to-block]*SCALE
            km_ps = ph2.enter_context(
                tc.tile_pool(name="km_ps", bufs=1, space="PSUM")
            )
            bk1s = sb2.tile([1, D], F32)
            nc.scalar.mul(bk1s[:, :], bk1[:, :], SCALE)
            ones_j = sb2.tile([1, NV], F32)
            nc.vector.memset(ones_j[:, :], 1.0)
            kmT = sb2.tile([128, T, NV], F32)
            for to in range(T):
                kmT_ps = km_ps.tile([128, NV], F32, tag="kmt", bufs=2)
                nc.tensor.matmul(
                    kmT_ps[:, :],
                    bk1s[:, to * 128 : (to + 1) * 128],
                    ones_j[:, :],
                    start=True,
                    stop=False,
                )
                for t in range(T):
                    nc.tensor.matmul(
                        kmT_ps[:, :],
                        wk_sb[:, t, to * 128 : (to + 1) * 128],
                        xsf[:, t, :],
                        start=False,
                        stop=(t == T - 1),
                    )
                nc.scalar.copy(out=kmT[:, to, :], in_=kmT_ps[:, :])

            # kmH: block-diagonal head-masked copy of kmT.
            # kmH[p, tk, h*NV+j] = kmT[p, tk, j] if head(tk*128+p)==h else 0
            kmH = sb2.tile([128, T, H * NV], F32)
            nc.vector.memset(kmH[:, :, :], 0.0)
            for h in range(H):
                po = (h % 2) * 64
                th = h // 2
                nc.vector.tensor_copy(
                    kmH[po : po + 64, th, h * NV : (h + 1) * NV],
                    kmT[po : po + 64, th, :],
                )

            # wqT via PE transpose of wq_sb
            wqT = sb2.tile([128, T, D], F32)
            tp2 = ph2.enter_context(
                tc.tile_pool(name="tp2", bufs=2, space="PSUM")
            )
            for tr in range(T):
                for tcol in range(T):
                    tp = tp2.tile([128, 128], F32, tag="tpq")
                    nc.tensor.transpose(
                        tp[:, :],
                        wq_sb[:, tr, tcol * 128 : (tcol + 1) * 128],
                        ident_f32[:, :],
                    )
                    nc.scalar.copy(
                        out=wqT[:, tcol, tr * 128 : (tr + 1) * 128], in_=tp[:, :]
                    )

            # wqm[:, td, (h,j)] = sum_hd WqT[hd, td-block] * kmH[hd, (h,j)]
            wqm = consts.tile([128, T, H * NV], mm_dt)
            bqm = consts.tile([1, H * NV], mm_dt)
            for td in range(T):
                wqm_ps = km_ps.tile([128, H * NV], F32, tag="wqm", bufs=2)
                for tk in range(T):
                    nc.tensor.matmul(
                        wqm_ps[:, :],
                        wqT[:, tk, td * 128 : (td + 1) * 128],
                        kmH[:, tk, :],
                        start=(tk == 0),
                        stop=(tk == T - 1),
                    )
                nc.scalar.copy(out=wqm[:, td, :], in_=wqm_ps[:, :])
            bqm_ps = km_ps.tile([1, H * NV], F32, tag="bias_ps", bufs=2)
            for tk in range(T):
                nc.tensor.matmul(
                    bqm_ps[:, :],
                    bqT[:, tk : tk + 1],
                    kmH[:, tk, :],
                    start=(tk == 0),
                    stop=(tk == T - 1),
                )
            nc.scalar.copy(out=bqm[:, :], in_=bqm_ps[:, :])

            # bo' = bv @ Wo + bo
            bop = consts.tile([1, D], mm_dt)
            bop_ps = km_ps.tile([1, D], F32, tag="bias_ps", bufs=2)
            nc.tensor.matmul(
                bop_ps[:, :], one1_f32[:, :], bo1[:, :], start=True, stop=False
            )
            for t in range(T):
                nc.tensor.matmul(
                    bop_ps[:, :],
                    _mm_ap(bvT[:, t : t + 1], cfg),
                    _mm_ap(wo_sb[:, t, :], cfg),
                    start=False,
                    stop=(t == T - 1),
                )
            nc.scalar.copy(out=bop[:, :], in_=bop_ps[:, :])

        # ---- phase 4: main loop over s-blocks ----
        sc_ps = ctx.enter_context(tc.tile_pool(name="sc_ps", bufs=1, space="PSUM"))
        v_ps = ctx.enter_context(tc.tile_pool(name="v_ps", bufs=2, space="PSUM"))
        tp_ps2 = ctx.enter_context(
            tc.tile_pool(name="tp_ps2", bufs=1, space="PSUM")
        )
        y_ps = ctx.enter_context(tc.tile_pool(name="y_ps", bufs=2, space="PSUM"))
        xtb_pool = ctx.enter_context(tc.tile_pool(name="xtb", bufs=2))
        xtp_ps = ctx.enter_context(
            tc.tile_pool(name="xtp_ps", bufs=2, space="PSUM")
        )
        vw_pool = ctx.enter_context(tc.tile_pool(name="vw", bufs=10))
        sm_pool = ctx.enter_context(tc.tile_pool(name="sm", bufs=3))
        acc_pool = ctx.enter_context(tc.tile_pool(name="acc", bufs=10))
        yo_pool = ctx.enter_context(tc.tile_pool(name="yo", bufs=3))

        for blk in range(SH):
            rs = slice(blk * 128, (blk + 1) * 128)
            # transpose this block of x for all views: xTb[d%128, t, i, s']
            xTb = xtb_pool.tile([128, T, NV, 128], mm_dt, tag="xtb")
            for i in range(NV):
                for t in range(T):
                    tp = xtp_ps.tile([128, 128], mm_dt, tag="xtp")
                    nc.tensor.transpose(
                        tp[:, :],
                        stg[:, i, blk, t * 128 : (t + 1) * 128],
                        ident_mm[:, :],
                    )
                    nc.scalar.copy(out=xTb[:, t, i, :], in_=tp[:, :])
            v_sb = []
            w_sb = []
            for i in range(NV):
                scp = sc_ps.tile([128, H * NV], F32, tag="scp")
                nc.tensor.matmul(
                    scp[:, :],
                    _mm_ap(ones_mm[:, :], cfg),
                    _mm_ap(bqm[:, :], cfg),
                    start=True,
                    stop=False,
                )
                vp = v_ps.tile([128, D], F32, tag="vp")
                for t in range(T):
                    nc.tensor.matmul(
                        vp[:, :],
                        _mm_ap(xTb[:, t, i, :], cfg),
                        _mm_ap(wv_sb[:, t, :], cfg),
                        start=(t == 0),
                        stop=(t == T - 1),
                    )
                    nc.tensor.matmul(
                        scp[:, :],
                        _mm_ap(xTb[:, t, i, :], cfg),
                        _mm_ap(wqm[:, t, :], cfg),
                        start=False,
                        stop=(t == T - 1),
                    )
                vt = vw_pool.tile([128, D], mix_dt, tag="v")
                nc.scalar.copy(out=vt[:, :], in_=vp[:, :])
                v_sb.append(vt)

                e_sb = sm_pool.tile([128, H * NV], F32, tag="e")
                nc.scalar.activation(e_sb[:, :], scp[:, :], Exp)
                z = sm_pool.tile([128, H], F32, tag="z")
                nc.vector.tensor_reduce(
                    z[:, :],
                    e_sb[:, :].rearrange("p (h j) -> p h j", h=H),
                    axis=mybir.AxisListType.X,
                    op=ADD,
                )
                rz = sm_pool.tile([128, H], F32, tag="rz")
                nc.vector.reciprocal(rz[:, :], z[:, :])
                wt = vw_pool.tile([128, H * NV], mix_dt, tag="w")
                nc.vector.tensor_tensor(
                    out=wt[:, :].rearrange("p (h j) -> p h j", h=H),
                    in0=e_sb[:, :].rearrange("p (h j) -> p h j", h=H),
                    in1=rz[:, :].unsqueeze(2).broadcast_to([128, H, NV]),
                    op=MULT,
                )
                w_sb.append(wt)

            # cross-view mix: acc[i] = sum_j w[i,(h,j)] * v[j,(h,:)]
            accs = []
            for i in range(NV):
                acc = acc_pool.tile([128, D], mix_dt, tag="acc")
                accs.append(acc)
                if cfg["mix_impl"] == "stt":
                    for j in range(NV):
                        for h in range(H):
                            hs = slice(h * HD, (h + 1) * HD)
                            wsc = w_sb[i][:, h * NV + j : h * NV + j + 1]
                            if j == 0:
                                nc.vector.tensor_scalar(
                                    acc[:, hs], v_sb[j][:, hs], wsc, None, op0=MULT
                                )
                            else:
                                nc.vector.scalar_tensor_tensor(
                                    out=acc[:, hs],
                                    in0=v_sb[j][:, hs],
                                    scalar=wsc,
                                    in1=acc[:, hs],
                                    op0=MULT,
                                    op1=ADD,
                                )
                else:
                    wr = w_sb[i][:, :].rearrange("p (h j) -> p h j", h=H)
                    for j in range(NV):
                        vj = v_sb[j][:, :].rearrange("p (h d) -> p h d", h=H)
                        wj = wr[:, :, j : j + 1].broadcast_to([128, H, HD])
                        if j == 0:
                            nc.vector.tensor_tensor(
                                out=acc[:, :].rearrange("p (h d) -> p h d", h=H),
                                in0=vj,
                                in1=wj,
                                op=MULT,
                            )
                        else:
                            tmp = acc_pool.tile([128, D], mix_dt, tag="mixtmp")
                            nc.vector.tensor_tensor(
                                out=tmp[:, :].rearrange("p (h d) -> p h d", h=H),
                                in0=vj,
                                in1=wj,
                                op=MULT,
                            )
                            nc.vector.tensor_add(acc[:, :], acc[:, :], tmp[:, :])

            # y projection (transpose acc, then rows-on-psum matmul)
            for i in range(NV):
                accT = yo_pool.tile([128, T, 128], mm_dt, tag="accT")
                for c in range(T):
                    tp = tp_ps2.tile([128, 128], mix_dt, tag="tpy")
                    nc.tensor.transpose(
                        tp[:, :],
                        accs[i][:, c * 128 : (c + 1) * 128],
                        ident_mix[:, :],
                    )
                    nc.scalar.copy(out=accT[:, c, :], in_=tp[:, :])
                yp = y_ps.tile([128, D], F32, tag="yp")
                nc.tensor.matmul(
                    yp[:, :],
                    _mm_ap(ones_mm[:, :], cfg),
                    _mm_ap(bop[:, :], cfg),
                    start=True,
                    stop=False,
                )
                for c in range(T):
                    nc.tensor.matmul(
                        yp[:, :],
                        _mm_ap(accT[:, c, :], cfg),
                        _mm_ap(wo_sb[:, c, :], cfg),
                        start=False,
                        stop=(c == T - 1),
                    )
                y_sb = yo_pool.tile([128, D], F32, tag="ysb")
                nc.scalar.copy(out=y_sb[:, :], in_=yp[:, :])
                nc.sync.dma_start(out=y[i, rs, :], in_=y_sb[:, :])


_BUILD_CACHE = {}
LAST_RESULT = None


def _get_nc(cfg):
    key = tuple(sorted(cfg.items()))
    if key not in _BUILD_CACHE:
        _BUILD_CACHE[key] = build_kernel(cfg)
    return _BUILD_CACHE[key]


def kernel(**inputs):
    global LAST_RESULT
    cfg = _cfg()
    nc = _get_nc(cfg)

    x = np.asarray(inputs["x"], dtype=np.float32)
    weights = {
        k: np.ascontiguousarray(np.asarray(inputs[k], dtype=np.float32))
        for k in ["Wq", "bq", "Wk", "bk", "Wv", "bv", "Wo", "bo"]
    }

    in_maps = []
    for c in range(NCORES):
        b, half = c // 2, c % 2
        xs = np.ascontiguousarray(
            x[b * NV : (b + 1) * NV, half * SPC : (half + 1) * SPC, :]
        )
        m = np.zeros((128, B), dtype=np.float32)
        m[:, b] = 1.0
        im = {"x": xs, "mask": m}
        im.update(weights)
        in_maps.append(im)

    res = run_bass_kernel_spmd(
        nc,
        in_maps,
        core_ids=list(range(NCORES)),
        trace=bool(int(os.environ.get("CVA_TRACE", "0"))),
    )
    LAST_RESULT = res

    out = np.empty((B * NV, S, D), dtype=np.float32)
    for c in range(NCORES):
        b, half = c // 2, c % 2
        out[b * NV : (b + 1) * NV, half * SPC : (half + 1) * SPC, :] = res.results[
            c
        ]["y"]
    return out



# revision 4
# speedup vs baseline: 1.0216x; 1.0216x over previous
"""CrossViewAttention Trainium2 Bass kernel.

Math (per batch-group b of NV=8 views, identical to reference):
  kmean[b,j]   = mean_s(x[b,j,s,:]) @ Wk + bk            (linearity of mean)
  scores       = (x @ Wq + bq) . kmean * scale
               = x @ Wqm[b] + bqm[b]                      (fold Wq into kmean)
  w            = softmax_j(scores)
  out[b,i,s]   = sum_j w[b,i,j,h,s] * v[b,j,s,(h,:)],  v = x @ Wv   (+bv folded)
  y            = out @ Wo + (bv @ Wo + bo)                (softmax sums to 1)

Sharding: 8 cores = 4 batch-groups x 2 S-halves. The only cross-core
quantity is the per-(b,view) feature-sum of x over S, exchanged with a
tiny AllReduce ([4,8,512] f32, full 8-core group with one-hot masking so
the SPMD program needs no core-dependent addressing).

Cross-view mix on TensorE (not VectorE): stack all 8 views' v-rows for a
group of 16 positions onto the 128 partitions (K=(j,sigma16)).  The
moving operand ED[(j,sigma), (h,i,s')] = w[i,j,h,s'] * delta(sigma,
s'%16) is built per query-view with one selection matmul per head
(partition-broadcast of the transposed softmax weights) plus one masked
PSUM->SBUF tensor_tensor on VectorE.  One matmul per (head-pair,
position-group) then computes U^T[(h2,d), (h2,i,s')] directly in the
transposed layout the Wo projection needs, so the old per-(i,j) VectorE
multiply-add chain AND the acc PE-transposes both disappear.
"""

import os
import numpy as np
from contextlib import ExitStack

import concourse.bass as bass
import concourse.bacc as bacc
import concourse.tile as tile
import concourse.mybir as mybir
from concourse.bass_utils import run_bass_kernel_spmd
from concourse.masks import make_identity

# ---- problem constants (hardcoded; kernel.py must be self-contained) ----
B, NV, S, D, H, HD = 4, 8, 2048, 512, 8, 64
NCORES = 8
# CVA_SPC: shrink per-core positions for simulator debugging only
SPC = int(os.environ.get("CVA_SPC", S // 2))
SH = SPC // 128         # s-blocks of 128
T = D // 128            # 4 contraction tiles
HN = H * NV             # 64 (h,j) score columns
SG = 16                 # positions per sigma-group (128 / NV)
G = 128 // SG           # 8 sigma-groups per s-block
SCALE = HD ** -0.5

F32 = mybir.dt.float32
BF16 = mybir.dt.bfloat16

REPLICA_GROUPS = [list(range(NCORES))]


def _cfg():
    return dict(
        reps=int(os.environ.get("CVA_REPS", "1")),        # timing: body reps
    )


def build_kernel(cfg):
    nc = bacc.Bacc(
        "TRN2", target_bir_lowering=False, debug=False, num_devices=NCORES
    )

    x = nc.dram_tensor("x", [NV, SPC, D], F32, kind="ExternalInput").ap()
    Wq = nc.dram_tensor("Wq", [D, D], F32, kind="ExternalInput").ap()
    bq = nc.dram_tensor("bq", [D], F32, kind="ExternalInput").ap()
    Wk = nc.dram_tensor("Wk", [D, D], F32, kind="ExternalInput").ap()
    bk = nc.dram_tensor("bk", [D], F32, kind="ExternalInput").ap()
    Wv = nc.dram_tensor("Wv", [D, D], F32, kind="ExternalInput").ap()
    bv = nc.dram_tensor("bv", [D], F32, kind="ExternalInput").ap()
    Wo = nc.dram_tensor("Wo", [D, D], F32, kind="ExternalInput").ap()
    bo = nc.dram_tensor("bo", [D], F32, kind="ExternalInput").ap()
    mask = nc.dram_tensor("mask", [128, B], F32, kind="ExternalInput").ap()
    sel = nc.dram_tensor("sel", [HN, H, 128], F32, kind="ExternalInput").ap()
    m16 = nc.dram_tensor("m16", [128, 128], F32, kind="ExternalInput").ap()
    y = nc.dram_tensor("y", [NV, SPC, D], F32, kind="ExternalOutput").ap()

    with tile.TileContext(nc) as tc:
        for _rep in range(cfg.get("reps", 1)):
            _body(tc, x, Wq, bq, Wk, bk, Wv, bv, Wo, bo, mask, sel, m16, y)

    nc.compile()
    return nc


def _body(tc, x, Wq, bq, Wk, bk, Wv, bv, Wo, bo, mask, sel, m16, y):
    nc = tc.nc
    Exp = mybir.ActivationFunctionType.Exp
    ADD = mybir.AluOpType.add
    MULT = mybir.AluOpType.mult

    ctx = ExitStack()
    with ctx:
        consts = ctx.enter_context(tc.tile_pool(name="consts", bufs=1))

        # ---- constants / weights resident in SBUF ----
        ident_f32 = consts.tile([128, 128], F32)
        make_identity(nc, ident_f32[:, :])
        ident_bf16 = consts.tile([128, 128], BF16)
        make_identity(nc, ident_bf16[:, :])
        ones_mm = consts.tile([1, 128], BF16)
        nc.vector.memset(ones_mm[:, :], 1.0)
        # onehots[:, i, :] = column-i one-hot [128, NV] (xsum stationary)
        onehots = consts.tile([128, NV, NV], BF16)
        nc.vector.memset(onehots[:, :, :], 0.0)
        for i in range(NV):
            nc.vector.memset(onehots[:, i, i : i + 1], 1.0)
        one1_f32 = consts.tile([1, 1], F32)
        nc.vector.memset(one1_f32[:, :], 1.0)

        # moving-operand weights [d_in(128), t, d_out(512)]
        wv_sb = consts.tile([128, T, D], BF16)
        wo_sb = consts.tile([128, T, D], BF16)
        nc.gpsimd.dma_start(
            out=wv_sb[:, :, :], in_=Wv.rearrange("(t p) n -> p t n", p=128)
        )
        nc.gpsimd.dma_start(
            out=wo_sb[:, :, :], in_=Wo.rearrange("(t p) n -> p t n", p=128)
        )
        bk1 = consts.tile([1, D], F32)
        nc.sync.dma_start(out=bk1[:, :], in_=bk.unsqueeze(0))
        bo1 = consts.tile([1, D], F32)
        nc.sync.dma_start(out=bo1[:, :], in_=bo.unsqueeze(0))
        bqT = consts.tile([128, T], F32)
        nc.sync.dma_start(out=bqT[:, :], in_=bq.rearrange("(t p) -> p t", p=128))
        bvT = consts.tile([128, T], BF16)
        nc.gpsimd.dma_start(out=bvT[:, :], in_=bv.rearrange("(t p) -> p t", p=128))
        mask_sb = consts.tile([128, B], F32)
        nc.sync.dma_start(out=mask_sb[:, :], in_=mask)
        # mix constants: per-head j-selection matrices + sigma-diag mask
        sel_sb = consts.tile([HN, H, 128], BF16)
        nc.gpsimd.dma_start(out=sel_sb[:, :, :], in_=sel)
        m16_sb = consts.tile([128, 128], BF16)
        nc.gpsimd.dma_start(out=m16_sb[:, :], in_=m16)

        # resident row-major staged activations [s%128, view, s_blk, d]
        stg = consts.tile([128, NV, SH, D], BF16)

        # ---- phase 1: load x + per-view feature sums ----
        ph1 = ExitStack()
        with ph1:
            xs_ps = ph1.enter_context(
                tc.tile_pool(name="xs_ps", bufs=1, space="PSUM")
            )
            xsum_ps = xs_ps.tile([NV, D], F32)
            for i in range(NV):
                nc.gpsimd.dma_start(
                    out=stg[:, i, :, :],
                    in_=x[i].rearrange("(sh p) d -> p sh d", p=128),
                )
                # feature sums via one-hot matmul; one accumulation group
                # over all views and s-blocks into xsum_ps[view, :]
                for sh in range(SH):
                    nc.tensor.matmul(
                        xsum_ps[:, :],
                        onehots[:, i, :],
                        stg[:, i, sh, :],
                        start=(i == 0 and sh == 0),
                        stop=(i == NV - 1 and sh == SH - 1),
                    )
            xsum_sb = consts.tile([NV, D], F32)
            nc.scalar.copy(out=xsum_sb[:, :], in_=xsum_ps[:, :])

        # ---- phase 2: exchange partial sums (masked full-group AllReduce) ----
        wqm = consts.tile([128, T, HN], BF16)
        bqm = consts.tile([1, HN], BF16)
        bop = consts.tile([1, D], BF16)
        ph2 = ExitStack()
        with ph2:
            dram = ph2.enter_context(
                tc.tile_pool(name="dram", bufs=1, space="DRAM")
            )
            sb2 = ph2.enter_context(tc.tile_pool(name="sb2", bufs=1))
            xsum4 = sb2.tile([NV, B, D], F32)
            for bb in range(B):
                nc.vector.tensor_scalar(
                    xsum4[:, bb, :],
                    xsum_sb[:, :],
                    mask_sb[0:NV, bb : bb + 1],
                    None,
                    op0=MULT,
                )
            cc_in = dram.tile([B, NV, D], F32)
            cc_out = dram.tile([B, NV, D], F32, addr_space="Shared")
            nc.sync.dma_start(
                out=cc_in[:, :, :].rearrange("b j d -> j b d"),
                in_=xsum4[:, :, :],
            )
            nc.gpsimd.collective_compute(
                "AllReduce",
                ADD,
                replica_groups=REPLICA_GROUPS,
                ins=[cc_in[:, :, :]],
                outs=[cc_out[:, :, :]],
            )
            # pull back all 4 groups, mask-select ours, scale by 1/S * scale
            xsf4 = sb2.tile([128, B, T, NV], F32)
            for bb in range(B):
                for t in range(T):
                    nc.sync.dma_start(
                        out=xsf4[:, bb, t, :],
                        in_=cc_out[bb, :, t * 128 : (t + 1) * 128].rearrange(
                            "j p -> p j"
                        ),
                    )
            xsf = sb2.tile([128, T, NV], F32)
            nc.vector.tensor_scalar(
                xsf[:, :, :],
                xsf4[:, 0, :, :],
                mask_sb[:, 0:1],
                None,
                op0=MULT,
            )
            for bb in range(1, B):
                nc.vector.scalar_tensor_tensor(
                    out=xsf[:, :, :],
                    in0=xsf4[:, bb, :, :],
                    scalar=mask_sb[:, bb : bb + 1],
                    in1=xsf[:, :, :],
                    op0=MULT,
                    op1=ADD,
                )
            nc.vector.tensor_scalar(
                xsf[:, :, :], xsf[:, :, :], SCALE / (2 * SPC), None, op0=MULT
            )

            # ---- phase 3: kmeanT, Wqm, bqm, bo' ----
            # kmT[:, to, j] = kmean[j, to*128+p] * SCALE
            #   = (xsum[j]*SCALE/S) @ Wk[:, to-block] + bk[to-block]*SCALE
            wk_sb = sb2.tile([128, T, D], F32)
            nc.sync.dma_start(
                out=wk_sb[:, :, :], in_=Wk.rearrange("(t p) n -> p t n", p=128)
            )
            wq_sb = sb2.tile([128, T, D], F32)
            nc.sync.dma_start(
                out=wq_sb[:, :, :], in_=Wq.rearrange("(t p) n -> p t n", p=128)
            )
            km_ps = ph2.enter_context(
                tc.tile_pool(name="km_ps", bufs=1, space="PSUM")
            )
            bk1s = sb2.tile([1, D], F32)
            nc.scalar.mul(bk1s[:, :], bk1[:, :], SCALE)
            ones_j = sb2.tile([1, NV], F32)
            nc.vector.memset(ones_j[:, :], 1.0)
            kmT = sb2.tile([128, T, NV], F32)
            for to in range(T):
                kmT_ps = km_ps.tile([128, NV], F32, tag="kmt", bufs=2)
                nc.tensor.matmul(
                    kmT_ps[:, :],
                    bk1s[:, to * 128 : (to + 1) * 128],
                    ones_j[:, :],
                    start=True,
                    stop=False,
                )
                for t in range(T):
                    nc.tensor.matmul(
                        kmT_ps[:, :],
                        wk_sb[:, t, to * 128 : (to + 1) * 128],
                        xsf[:, t, :],
                        start=False,
                        stop=(t == T - 1),
                    )
                nc.scalar.copy(out=kmT[:, to, :], in_=kmT_ps[:, :])

            # kmH: block-diagonal head-masked copy of kmT.
            # kmH[p, tk, h*NV+j] = kmT[p, tk, j] if head(tk*128+p)==h else 0
            kmH = sb2.tile([128, T, HN], F32)
            nc.vector.memset(kmH[:, :, :], 0.0)
            for h in range(H):
                po = (h % 2) * 64
                th = h // 2
                nc.vector.tensor_copy(
                    kmH[po : po + 64, th, h * NV : (h + 1) * NV],
                    kmT[po : po + 64, th, :],
                )

            # wqT via PE transpose of wq_sb
            wqT = sb2.tile([128, T, D], F32)
            tp2 = ph2.enter_context(
                tc.tile_pool(name="tp2", bufs=2, space="PSUM")
            )
            for tr in range(T):
                for tcol in range(T):
                    tp = tp2.tile([128, 128], F32, tag="tpq")
                    nc.tensor.transpose(
                        tp[:, :],
                        wq_sb[:, tr, tcol * 128 : (tcol + 1) * 128],
                        ident_f32[:, :],
                    )
                    nc.scalar.copy(
                        out=wqT[:, tcol, tr * 128 : (tr + 1) * 128], in_=tp[:, :]
                    )

            # wqm[:, td, (h,j)] = sum_hd WqT[hd, td-block] * kmH[hd, (h,j)]
            for td in range(T):
                wqm_ps = km_ps.tile([128, HN], F32, tag="wqm", bufs=2)
                for tk in range(T):
                    nc.tensor.matmul(
                        wqm_ps[:, :],
                        wqT[:, tk, td * 128 : (td + 1) * 128],
                        kmH[:, tk, :],
                        start=(tk == 0),
                        stop=(tk == T - 1),
                    )
                nc.scalar.copy(out=wqm[:, td, :], in_=wqm_ps[:, :])
            bqm_ps = km_ps.tile([1, HN], F32, tag="bias_ps", bufs=2)
            for tk in range(T):
                nc.tensor.matmul(
                    bqm_ps[:, :],
                    bqT[:, tk : tk + 1],
                    kmH[:, tk, :],
                    start=(tk == 0),
                    stop=(tk == T - 1),
                )
            nc.scalar.copy(out=bqm[:, :], in_=bqm_ps[:, :])

            # bo' = bv @ Wo + bo
            bop_ps = km_ps.tile([1, D], F32, tag="bias_ps", bufs=2)
            nc.tensor.matmul(
                bop_ps[:, :], one1_f32[:, :], bo1[:, :], start=True, stop=False
            )
            for t in range(T):
                nc.tensor.matmul(
                    bop_ps[:, :],
                    bvT[:, t : t + 1],
                    wo_sb[:, t, :],
                    start=False,
                    stop=(t == T - 1),
                )
            nc.scalar.copy(out=bop[:, :], in_=bop_ps[:, :])

        # ---- phase 4: main loop over s-blocks ----
        # PSUM budget (8 banks): small x2 + v x2 + E x1(2 banks) + mixy x2
        small_ps = ctx.enter_context(
            tc.tile_pool(name="small_ps", bufs=2, space="PSUM")
        )
        v_ps = ctx.enter_context(tc.tile_pool(name="v_ps", bufs=2, space="PSUM"))
        e_psum = ctx.enter_context(
            tc.tile_pool(name="e_psum", bufs=1, space="PSUM")
        )
        mixy_ps = ctx.enter_context(
            tc.tile_pool(name="mixy_ps", bufs=2, space="PSUM")
        )
        xtb_pool = ctx.enter_context(tc.tile_pool(name="xtb", bufs=2))
        vs_pool = ctx.enter_context(tc.tile_pool(name="vs", bufs=2))
        ed_pool = ctx.enter_context(tc.tile_pool(name="ed", bufs=2))
        acc_pool = ctx.enter_context(tc.tile_pool(name="acc", bufs=2))
        vw_pool = ctx.enter_context(tc.tile_pool(name="vw", bufs=2))
        sm_pool = ctx.enter_context(tc.tile_pool(name="sm", bufs=2))
        yo_pool = ctx.enter_context(tc.tile_pool(name="yo", bufs=3))

        for blk in range(SH):
            rs = slice(blk * 128, (blk + 1) * 128)
            # transpose this block of x for all views: xTb[d%128, i, t, s']
            xTb = xtb_pool.tile([128, NV, T, 128], BF16, tag="xtb")
            for i in range(NV):
                xtp = small_ps.tile([128, T, 128], BF16, tag="small")
                for t in range(T):
                    nc.tensor.transpose(
                        xtp[:, t, :],
                        stg[:, i, blk, t * 128 : (t + 1) * 128],
                        ident_bf16[:, :],
                    )
                if i % 2 == 0:
                    nc.scalar.copy(out=xTb[:, i, :, :], in_=xtp[:, :, :])
                else:
                    nc.vector.tensor_copy(xTb[:, i, :, :], xtp[:, :, :])

            # per view: v matmul + scores + softmax + ED build + v stacking
            vstack = vs_pool.tile([128, G, D], BF16, tag="vs")
            ED = ed_pool.tile([128, NV, H, 128], BF16, tag="ed")
            for i in range(NV):
                vp = v_ps.tile([128, D], F32, tag="vp")
                scp = small_ps.tile([128, HN], F32, tag="small")
                nc.tensor.matmul(
                    scp[:, :], ones_mm[:, :], bqm[:, :], start=True, stop=False
                )
                for t in range(T):
                    nc.tensor.matmul(
                        vp[:, :],
                        xTb[:, i, t, :],
                        wv_sb[:, t, :],
                        start=(t == 0),
                        stop=(t == T - 1),
                    )
                    nc.tensor.matmul(
                        scp[:, :],
                        xTb[:, i, t, :],
                        wqm[:, t, :],
                        start=False,
                        stop=(t == T - 1),
                    )
                vt = vw_pool.tile([128, D], BF16, tag="v_sb")
                nc.scalar.copy(out=vt[:, :], in_=vp[:, :])
                # stack view i's rows into the (j, sigma) partition layout;
                # one DMA per sigma-group (SBUF partition ranges must stay
                # the leading AP dim on both sides)
                for g in range(G):
                    nc.sync.dma_start(
                        out=vstack[i * SG : (i + 1) * SG, g, :],
                        in_=vt[g * SG : (g + 1) * SG, :],
                    )

                # softmax over key-view axis j (per head)
                e_sb = sm_pool.tile([128, HN], F32, tag="e")
                nc.scalar.activation(e_sb[:, :], scp[:, :], Exp)
                z = sm_pool.tile([128, H], F32, tag="z")
                nc.vector.tensor_reduce(
                    z[:, :],
                    e_sb[:, :].rearrange("p (h j) -> p h j", h=H),
                    axis=mybir.AxisListType.X,
                    op=ADD,
                )
                rz = sm_pool.tile([128, H], F32, tag="rz")
                nc.vector.reciprocal(rz[:, :], z[:, :])
                wt = sm_pool.tile([128, HN], BF16, tag="w")
                nc.vector.tensor_tensor(
                    out=wt[:, :].rearrange("p (h j) -> p h j", h=H),
                    in0=e_sb[:, :].rearrange("p (h j) -> p h j", h=H),
                    in1=rz[:, :].unsqueeze(2).broadcast_to([128, H, NV]),
                    op=MULT,
                )
                # transpose w -> wT[(h,j), s']
                wTp = small_ps.tile([HN, 128], BF16, tag="small")
                nc.tensor.transpose(wTp[:, :], wt[:, :], ident_bf16[:, :])
                wT = sm_pool.tile([HN, 128], BF16, tag="wT")
                nc.scalar.copy(out=wT[:, :], in_=wTp[:, :])
                # partition-broadcast wT rows into (j, sigma) layout, per head
                E_ps = e_psum.tile([128, H, 128], F32, tag="E")
                for h in range(H):
                    nc.tensor.matmul(
                        E_ps[:, h, :],
                        sel_sb[:, h, :],
                        wT[:, :],
                        start=True,
                        stop=True,
                    )
                # masked evac: ED[(j,sigma), i, h, s'] = E_ps * delta(sigma, s'%16)
                nc.vector.tensor_tensor(
                    out=ED[:, i, :, :],
                    in0=E_ps[:, :, :],
                    in1=m16_sb[:, :].unsqueeze(1).broadcast_to([128, H, 128]),
                    op=MULT,
                )

            # cross-view mix on TensorE: per (head-pair, sigma-group-pair)
            # out[(h2,d), (h2',i,s')] -- keep the h2==h2' diagonal blocks
            ACC = acc_pool.tile([128, T, NV, 128], BF16, tag="acc")
            for p in range(T):
                for g2 in range(G // 2):
                    mp = mixy_ps.tile([128, 2, 256], F32, tag="mixy")
                    for gl in range(2):
                        g = g2 * 2 + gl
                        nc.tensor.matmul(
                            mp[:, gl, :],
                            vstack[:, g, p * 128 : (p + 1) * 128],
                            ED[:, :, 2 * p : 2 * p + 2, g * SG : (g + 1) * SG]
                            .rearrange("k i h s -> k h i s"),
                            start=True,
                            stop=True,
                        )
                    for h2 in range(2):
                        src = mp[
                            h2 * 64 : (h2 + 1) * 64, :, h2 * 128 : (h2 + 1) * 128
                        ].rearrange("p g (i s) -> p g i s", i=NV)
                        dst = ACC[
                            h2 * 64 : (h2 + 1) * 64, p, :,
                            g2 * 32 : (g2 + 1) * 32,
                        ].rearrange("p i (g s) -> p g i s", g=2)
                        if (p + g2) % 2 == 0:
                            nc.scalar.copy(out=dst, in_=src)
                        else:
                            nc.vector.tensor_copy(dst, src)

            # y projection: rows already transposed, straight to Wo matmul
            for i in range(NV):
                yp = mixy_ps.tile([128, D], F32, tag="mixy")
                nc.tensor.matmul(
                    yp[:, :], ones_mm[:, :], bop[:, :], start=True, stop=False
                )
                for p in range(T):
                    nc.tensor.matmul(
                        yp[:, :],
                        ACC[:, p, i, :],
                        wo_sb[:, p, :],
                        start=False,
                        stop=(p == T - 1),
                    )
                y_sb = yo_pool.tile([128, D], F32, tag="ysb")
                nc.scalar.copy(out=y_sb[:, :], in_=yp[:, :])
                nc.sync.dma_start(out=y[i, rs, :], in_=y_sb[:, :])


_BUILD_CACHE = {}
LAST_RESULT = None


def _get_nc(cfg):
    key = tuple(sorted(cfg.items()))
    if key not in _BUILD_CACHE:
        _BUILD_CACHE[key] = build_kernel(cfg)
    return _BUILD_CACHE[key]


def _host_constants():
    sel = np.zeros((HN, H, 128), dtype=np.float32)
    for h in range(H):
        for j in range(NV):
            sel[h * NV + j, h, j * SG : (j + 1) * SG] = 1.0
    m16 = np.zeros((128, 128), dtype=np.float32)
    for p in range(128):
        m16[p, p % SG :: SG] = 1.0
    return sel, m16


def kernel(**inputs):
    global LAST_RESULT
    cfg = _cfg()
    nc = _get_nc(cfg)

    x = np.asarray(inputs["x"], dtype=np.float32)
    weights = {
        k: np.ascontiguousarray(np.asarray(inputs[k], dtype=np.float32))
        for k in ["Wq", "bq", "Wk", "bk", "Wv", "bv", "Wo", "bo"]
    }
    sel, m16 = _host_constants()

    in_maps = []
    for c in range(NCORES):
        b, half = c // 2, c % 2
        xs = np.ascontiguousarray(
            x[b * NV : (b + 1) * NV, half * SPC : (half + 1) * SPC, :]
        )
        m = np.zeros((128, B), dtype=np.float32)
        m[:, b] = 1.0
        im = {"x": xs, "mask": m, "sel": sel, "m16": m16}
        im.update(weights)
        in_maps.append(im)

    res = run_bass_kernel_spmd(
        nc,
        in_maps,
        core_ids=list(range(NCORES)),
        trace=bool(int(os.environ.get("CVA_TRACE", "0"))),
    )
    LAST_RESULT = res

    out = np.empty((B * NV, S, D), dtype=np.float32)
    for c in range(NCORES):
        b, half = c // 2, c % 2
        out[b * NV : (b + 1) * NV, half * SPC : (half + 1) * SPC, :] = res.results[
            c
        ]["y"]
    return out
